# revision 44
# baseline (speedup 1.0000x reference)
"""Trainium2 Bass kernel for DeepKernelRegressionModel.

Math (per core, X sharded by rows across 8 cores):
  Xf = MLP(X), Yf = MLP(Y)                        (3-layer relu MLP, H=32)
  K[i,m] = exp(-|Xf_i - Yf_m|^2 / 2)
  out = (K @ Y_target) / (K @ 1)

Two identities remove all exponent augmentation:
  - the factor exp(-|Xf_i|^2/2) is constant per row i and cancels in the
    normalization, so it is never computed;
  - the factor exp(-|Yf_m|^2/2) is folded into the Z matrix instead of
    the exponent:  z'[m,:] = z[m,:] * exp(-|Yf_m|^2/2).
So mm1 is a pure K=32 product G'[m,i] = Yf_m . Xf_i (m on psum
partitions, 128 per tile), exp runs on ACT, and
  acc[t,i] += z'^T @ exp(G')
accumulates over all m.  A final tiny transpose + reciprocal normalizes.

Hardware constraints that shape the design (verified on neuronx-cc):
  - f32r matmuls (full-rate fp32, 1 cycle/row) are only legal with
    tile_position column 0, so every matmul keeps outputs at partition
    base 0: the MLPs run UNSTACKED on partitions 0-31 and the relus
    (DVE tensor_scalar add+max) pay free-size cost instead;
  - every f32r-matmul operand must be produced through an f32r-typed
    write (loads and relu/square outputs are bitcast);
  - GPSIMD cannot touch PSUM, so Pool only carries DMAs;
  - DMA cost is ~(per-partition bytes x 0.39ns), so X^T/Y^T are host-
    packed two 64-row bands high (W1 is duplicated to rows 64-127).

The kernel runs as one uniform per-chunk pipeline (512 Y rows each):
MLP -> relu (DVE) -> square -> norm matmul [1,512] -> 1-row PE
transposes into [p=m%128, mt] PSUM -> ACT exp -> DVE z-scale, then
immediately the main-loop groups (mm1 -> exp -> mm2) for that chunk's
four m-tiles on both i-chunks, so ACT streams exps almost from the
start and PE never waits on far-away phases.
"""

import os
import numpy as np
from contextlib import ExitStack

import concourse.bass as bass
import concourse.tile as tile
from concourse import bacc, mybir
from concourse.alu_op_type import AluOpType

FP = mybir.dt.float32
FPR = mybir.dt.float32r
AF = mybir.ActivationFunctionType

D, H, T = 64, 32, 8
ZP = 16     # Z columns per m-tile: 8 targets, 1 ones, 7 pad
N_CORES = 8

# packed-constant column layout (consts tile [128, CW])
C_W1, C_W2, C_W3, C_BS, C_ID, C_NH, C_ONE, C_ESH = 0, 32, 64, 96, 99, 115, 147, 148
CW = 160


def build_nc(n_sh, m_total, use_f32r=True, exp_group=2):
    """Build the Bass program for one core (SPMD: same program, all cores).

    n_sh: rows of X handled by this core. m_total: rows of Y (full).
    """
    assert n_sh % 512 == 0 and m_total % 2048 == 0
    MT = m_total // 128       # number of 128-row m-tiles
    NCH = m_total // 512      # number of 512-wide m-chunks
    IC = n_sh // 512          # i-chunks
    ICW = 512
    GA = exp_group            # m-tiles per exp group (4 % GA == 0)
    PY = m_total // 2048      # packed Y loads
    VX = n_sh // 512          # X 512-col chunks
    XROWS = 64 * min(VX, 2)
    XCOLS = 512 * ((VX + 1) // 2)

    def r(ap):
        return ap.bitcast(FPR) if use_f32r else ap

    nc = bacc.Bacc("TRN2", target_bir_lowering=False, debug=False,
                   num_devices=N_CORES)

    XTd = nc.dram_tensor("XT", [XROWS, XCOLS], FP, kind="ExternalInput").ap()
    YTd = nc.dram_tensor("YT", [128, 1024 * PY], FP, kind="ExternalInput").ap()
    ZMd = nc.dram_tensor("ZM", [128, MT * ZP], FP, kind="ExternalInput").ap()
    Cd = nc.dram_tensor("CONSTS", [128, CW], FP, kind="ExternalInput").ap()
    OUTd = nc.dram_tensor("out", [n_sh, T], FP, kind="ExternalOutput").ap()
    OUTr = OUTd.rearrange("(c q p) t -> c p q t", q=4, p=128)

    with tile.TileContext(nc) as tc, ExitStack() as ctx:
        const = ctx.enter_context(tc.tile_pool(name="const", bufs=1))
        big = ctx.enter_context(tc.tile_pool(name="big", bufs=1))
        scr = ctx.enter_context(tc.tile_pool(name="scr", bufs=1))
        actp = ctx.enter_context(tc.tile_pool(name="acts", bufs=4))
        epool = ctx.enter_context(tc.tile_pool(name="ebuf", bufs=5))
        finp = ctx.enter_context(tc.tile_pool(name="fin", bufs=2))

        # ---------------- constants + packed input loads ----------------
        cs = const.tile([128, CW], FP)
        nc.sync.dma_start(r(cs[:]), r(Cd[:]))
        w1 = cs[:, C_W1:C_W1 + H]          # W1 duplicated on rows 64-127
        w2 = cs[:, C_W2:C_W2 + H]
        w3 = cs[:, C_W3:C_W3 + H]
        bs = cs[:, C_BS:C_BS + 3]
        ident = cs[0:ZP, C_ID:C_ID + ZP]
        nh = cs[:, C_NH:C_NH + H]
        onec = cs[:, C_ONE:C_ONE + 1]
        esh = cs[:, C_ESH:C_ESH + 1]   # exponent shift -20

        xT = big.tile([XROWS, XCOLS], FP)
        nc.gpsimd.dma_start(r(xT[:]), r(XTd[:]))
        yT = big.tile([128, 1024 * PY], FP)
        nc.sync.dma_start(r(yT[:, 0:512]), r(YTd[:, 0:512]))
        nc.sync.dma_start(r(yT[:, 512:1024]), r(YTd[:, 512:1024]))
        for c in range(1, PY):
            eng = nc.sync if c % 2 == 0 else nc.gpsimd
            eng.dma_start(r(yT[:, 1024 * c:1024 * c + 1024]),
                          r(YTd[:, 1024 * c:1024 * c + 1024]))
        zt = const.tile([128, MT * ZP], FP)
        nc.gpsimd.dma_start(zt[:], ZMd[:])

        def arelu(out, in_, layer, p=128):     # ACT relu
            nc.scalar.activation(out, in_, AF.Relu,
                                 bias=bs[0:p, layer:layer + 1])

        def vrelu(out, in_, layer, p=128):     # DVE relu
            nc.vector.tensor_scalar(out, in_, bs[0:p, layer:layer + 1],
                                    0.0, AluOpType.add, AluOpType.max)

        yf = big.tile([H, m_total], FP)      # Yf^T
        sqy = big.tile([H, m_total], FP)     # Yf^T squared
        xf = big.tile([H, n_sh], FP)         # Xf^T
        sexp = scr.tile([128, MT], FP, tag="sexp")
        zts = const.tile([128, MT * ZP], FP)

        # -------- X MLP (rows 0-31), relus on ACT --------
        xs1 = scr.tile([H, n_sh], FP, tag="xs1")
        xs2 = scr.tile([H, n_sh], FP, tag="xs2")
        with tc.tile_pool(name="xpsum", bufs=2, space="PSUM") as xpp:
            for v in range(VX):
                Rv, cv = (v % 2) * 64, 512 * (v // 2)
                hp = xpp.tile([H, 512], FP, tag="hx")
                nc.tensor.matmul(hp[:], tile_position=(Rv, 0),
                                 lhsT=r(w1[Rv:Rv + D, :]),
                                 rhs=r(xT[Rv:Rv + D, cv:cv + 512]),
                                 start=True, stop=True, skip_group_check=True)
                arelu(r(xs1[:, 512 * v:512 * v + 512]), hp[:], 0, H)
            for v in range(VX):
                hp = xpp.tile([H, 512], FP, tag="hx")
                nc.tensor.matmul(hp[:], tile_position=(0, 0),
                                 lhsT=r(w2[0:H, :]),
                                 rhs=r(xs1[:, 512 * v:512 * v + 512]),
                                 start=True, stop=True, skip_group_check=True)
                arelu(r(xs2[:, 512 * v:512 * v + 512]), hp[:], 1, H)
            for v in range(VX):
                hp = xpp.tile([H, 512], FP, tag="hx")
                nc.tensor.matmul(hp[:], tile_position=(0, 0),
                                 lhsT=r(w3[0:H, :]),
                                 rhs=r(xs2[:, 512 * v:512 * v + 512]),
                                 start=True, stop=True, skip_group_check=True)
                arelu(r(xf[:, 512 * v:512 * v + 512]), hp[:], 2, H)

        # -------- uniform per-chunk pipeline --------
        accs = [None] * IC
        done = [0] * IC

        with (
            tc.tile_pool(name="hpool", bufs=1, space="PSUM") as hpool,
            tc.tile_pool(name="ring", bufs=1, space="PSUM") as ring,
            tc.tile_pool(name="gpool", bufs=2, space="PSUM") as gpool,
            tc.tile_pool(name="accp", bufs=2, space="PSUM") as apool,
        ):
            for ic in range(IC):
                accs[ic] = apool.tile([128, ICW], FP, tag="acc",
                                      name=f"acc{ic}")

            def crelu(ch, out, in_, layer):
                (arelu if ch == 0 else vrelu)(out, in_, layer, H)

            def chain_pre(ch):    # L1 + relu1
                c, local = ch // 4, ch % 4
                R = 64 * (local // 2)
                col = 1024 * c + 512 * (local % 2)
                hp = hpool.tile([H, 512], FP, tag="hp", name=f"h1_{ch}")
                nc.tensor.matmul(hp[:], tile_position=(R, 0),
                                 lhsT=r(w1[R:R + D, :]),
                                 rhs=r(yT[R:R + D, col:col + 512]),
                                 start=True, stop=True, skip_group_check=True)
                s1 = actp.tile([H, 512], FP, tag="hs")
                crelu(ch, r(s1[:]), hp[:], 0)
                return s1

            def chain_mid(ch, s1):  # L2 + relu2
                hp = hpool.tile([H, 512], FP, tag="hp", name=f"h2_{ch}")
                nc.tensor.matmul(hp[:], tile_position=(0, 0),
                                 lhsT=r(w2[0:H, :]), rhs=r(s1[:]),
                                 start=True, stop=True, skip_group_check=True)
                s2 = actp.tile([H, 512], FP, tag="hs")
                crelu(ch, r(s2[:]), hp[:], 1)
                return s2

            def chain_post(ch, s2):  # L3 + relu3 + square
                hp = hpool.tile([H, 512], FP, tag="hp", name=f"h3_{ch}")
                nc.tensor.matmul(hp[:], tile_position=(0, 0),
                                 lhsT=r(w3[0:H, :]), rhs=r(s2[:]),
                                 start=True, stop=True, skip_group_check=True)
                yfc = yf[:, 512 * ch:512 * ch + 512]
                crelu(ch, r(yfc), hp[:], 2)
                sqc = sqy[:, 512 * ch:512 * ch + 512]
                nc.vector.tensor_mul(r(sqc), yfc, yfc)

            def chain_norm(ch):  # norm -> scatter -> exp -> z-scale
                sqc = sqy[:, 512 * ch:512 * ch + 512]
                ynp = ring.tile([1, 512], FP, tag="rg", name=f"ynp{ch}")
                nc.tensor.matmul(ynp[:], tile_position=(0, 0),
                                 lhsT=r(nh[0:H, 0:1]), rhs=r(sqc),
                                 start=True, stop=True, skip_group_check=True)
                yns = actp.tile([1, 512], FP, tag="yns")
                nc.vector.tensor_copy(r(yns[:]), ynp[:])
                ntp = ring.tile([128, 4], FP, tag="rg", name=f"ntp{ch}")
                for b in range(4):
                    nc.tensor.matmul(
                        ntp[:, b:b + 1], tile_position=(0, 0),
                        lhsT=yns[0:1, 128 * b:128 * b + 128],
                        rhs=onec[0:1, :], is_transpose=True,
                        start=(b == 0), stop=(b == 3),
                        skip_group_check=True)
                nc.scalar.activation(r(sexp[:, 4 * ch:4 * ch + 4]),
                                     ntp[:], AF.Exp)
                lo = ZP * 4 * ch
                nc.vector.tensor_mul(
                    r(zts[:, lo:lo + 4 * ZP]).rearrange(
                        "p (m z) -> p m z", z=ZP),
                    zt[:, lo:lo + 4 * ZP].rearrange("p (m z) -> p m z", z=ZP),
                    sexp[:, 4 * ch:4 * ch + 4]
                        .rearrange("p m -> p m ()").broadcast_to([128, 4, ZP]))

            NGC = IC * (4 // GA)   # exp groups per chunk

            def group_fe(ch, j):  # mm1s + exp of the j-th group
                ic, g0 = j // (4 // GA), (j % (4 // GA)) * GA
                grp = [4 * ch + g0 + t for t in range(GA)]
                gp = gpool.tile([128, 512 * GA], FP, tag="g")
                for t, mt in enumerate(grp):
                    nc.tensor.matmul(
                        gp[:, 512 * t:512 * t + 512],
                        tile_position=(0, 0),
                        lhsT=r(yf[0:H, 128 * mt:128 * mt + 128]),
                        rhs=r(xf[0:H, ICW * ic:ICW * ic + ICW]),
                        start=True, stop=True)
                eb = epool.tile([128, 512 * GA], FP, tag="e")
                # constant shift cancels in the normalization; it centers the
                # exponent range so exp(Yf.Xf) cannot overflow for hot inputs
                nc.scalar.activation(r(eb[:]), gp[:], AF.Exp, bias=esh)
                return eb, grp, ic

            def group_be(fe):     # mm2 accumulations of a group
                eb, grp, ic = fe
                for t, mt in enumerate(grp):
                    nc.tensor.matmul(
                        accs[ic][0:ZP, :],
                        tile_position=(0, 0),
                        lhsT=r(zts[:, ZP * mt:ZP * mt + ZP]),
                        rhs=r(eb[:, 512 * t:512 * t + 512]),
                        start=(done[ic] == 0),
                        stop=(done[ic] == MT - 1),
                        skip_group_check=True)
                    done[ic] += 1

            def group(ch, j):
                group_be(group_fe(ch, j))

            def fold(ic):
                # fold 4 col-group accumulators via transpose-accumulate
                acc_s = finp.tile([ZP, ICW], FP, tag="accs")
                nc.vector.tensor_copy(acc_s[:], accs[ic][0:ZP, :])
                ot = apool.tile([128, 4 * ZP], FP, tag="acc",
                                name=f"ot{ic}")
                for q in range(4):
                    nc.tensor.matmul(
                        ot[:, ZP * q:ZP * q + ZP],
                        tile_position=(0, 0),
                        lhsT=acc_s[0:ZP, 128 * q:128 * q + 128],
                        rhs=ident,
                        is_transpose=True,
                        start=(q == 0), stop=(q == 3),
                        skip_group_check=True)
                resb = finp.tile([128, 4 * T], FP, tag="res")
                rec = finp.tile([128, 4], FP, tag="rec")
                otv = ot.rearrange("p (q z) -> p q z", z=ZP)
                nc.vector.reciprocal(rec[:], otv[:, :, T:T + 1])
                nc.vector.tensor_mul(
                    resb.rearrange("p (q t) -> p q t", q=4),
                    otv[:, :, 0:T],
                    rec[:].rearrange("p q -> p q ()").broadcast_to([128, 4, T]))
                nc.sync.dma_start(OUTr[ic],
                                  resb.rearrange("p (q t) -> p q t", q=4))

            # software pipeline: chunk ch+1's chain stages interleave with
            # chunk ch's four exp groups (PE stays in-order but never stalls)
            LK = 2 if NCH > 2 else 1      # chain lookahead depth
            for c0 in range(min(LK, NCH)):
                s1 = chain_pre(c0)
                s2 = chain_mid(c0, s1)
                chain_post(c0, s2)
                chain_norm(c0)
            st = {}
            for ch in range(NCH):
                nxt = ch + LK
                for j in range(max(4, NGC)):
                    if nxt < NCH and j < 4:
                        if j == 0:
                            st['s1'] = chain_pre(nxt)
                        elif j == 1:
                            st['s2'] = chain_mid(nxt, st['s1'])
                        elif j == 2:
                            chain_post(nxt, st['s2'])
                        else:
                            chain_norm(nxt)
                    if j < NGC:
                        group(ch, j)
                        if (IC > 1 and ch == NCH - 1
                                and j == NGC // 2 - 1):
                            fold(0)

            # ---- normalize + store (ic0 already folded mid-pipeline) ----
            for ic in range(0 if IC == 1 else 1, IC):
                fold(ic)
    nc.compile()
    return nc


def make_in_maps(X, Y, Y_target, W1, b1, W2, b2, W3, b3, n_cores=N_CORES):
    f = lambda a: np.ascontiguousarray(np.asarray(a, dtype=np.float32))
    X, Y, Y_target = f(X), f(Y), f(Y_target)
    W1, W2, W3 = f(W1), f(W2), f(W3)
    b1, b2, b3 = f(b1), f(b2), f(b3)
    m_total = Y.shape[0]
    n_sh = X.shape[0] // n_cores
    MT = m_total // 128
    Z = np.zeros((m_total, ZP), np.float32)
    Z[:, :T] = Y_target
    Z[:, T] = 1.0
    ZM = np.ascontiguousarray(
        Z.reshape(MT, 128, ZP).transpose(1, 0, 2).reshape(128, MT * ZP))
    C = np.zeros((128, CW), np.float32)
    C[:D, C_W1:C_W1 + H] = W1
    C[D:, C_W1:C_W1 + H] = W1
    C[:, C_W2:C_W2 + H] = np.tile(W2, (4, 1))
    C[:, C_W3:C_W3 + H] = np.tile(W3, (4, 1))
    C[:, C_BS:C_BS + 3] = np.stack(
        [np.tile(b1, 4), np.tile(b2, 4), np.tile(b3, 4)], axis=1)
    C[:ZP, C_ID:C_ID + ZP] = np.eye(ZP, dtype=np.float32)
    C[:, C_NH:C_NH + H] = -0.5
    C[:, C_ONE] = 1.0
    C[:, C_ESH] = -20.0

    def packX(A):  # [d, n]: 512-blocks alternate between two 64-row bands
        d, n = A.shape
        if (n // 512) % 2 != 0:
            return np.ascontiguousarray(A)
        return np.ascontiguousarray(
            A.reshape(d, n // 1024, 2, 512).transpose(2, 0, 1, 3)
             .reshape(2 * d, n // 2))

    def packY(A):  # [d, n]: per 2048-span, 1024-halves stack on row bands
        d, n = A.shape
        assert n % 2048 == 0
        return np.ascontiguousarray(
            A.reshape(d, n // 2048, 2, 1024).transpose(2, 0, 1, 3)
             .reshape(2 * d, n // 2))

    common = dict(YT=packY(Y.T), ZM=ZM, CONSTS=C)
    return [dict(common, XT=packX(X[c * n_sh:(c + 1) * n_sh].T))
            for c in range(n_cores)]


_NC_CACHE = {}


def _get_nc(n_sh, m_total):
    key = (n_sh, m_total)
    if key not in _NC_CACHE:
        use_f32r = os.environ.get("DKR_F32R", "1") == "1"
        _NC_CACHE[key] = build_nc(n_sh, m_total, use_f32r=use_f32r)
    return _NC_CACHE[key]


def kernel(X, Y, Y_target, W1, b1, W2, b2, W3, b3):
    from concourse.bass_utils import run_bass_kernel_spmd

    in_maps = make_in_maps(X, Y, Y_target, W1, b1, W2, b2, W3, b3)
    n_sh = np.asarray(X).shape[0] // N_CORES
    nc = _get_nc(n_sh, np.asarray(Y).shape[0])
    res = run_bass_kernel_spmd(nc, in_maps, core_ids=list(range(N_CORES)))
    return np.concatenate([res.results[c]["out"] for c in range(N_CORES)], axis=0)
